# revision 1
# baseline (speedup 1.0000x reference)
"""Distributed Trainium2 kernel for nn_CONNECT_86964497809993 (TGN-style
GNN message passing: last-event aggregation + GRU memory update + community
incidence matmul), sharded over 8 NeuronCores by node id.

Strategy (per sharding hint): nodes are block-sharded across the 8 cores.
Event routing ("last message per node" selection) is pure integer index
plumbing, done on the host during input sharding; each core then runs the
full FP pipeline for its 12 500 nodes on-device:
  - time encoding  tenc = cos(dt*w + b) via range-reduced Sin LUT
  - gates          gx + gh = [embs|embd|feat|tenc|1] @ [W_ih;bias] + mem @ W_hh
  - GRU            r,z = sigmoid;  n = tanh(xn + r*hn);  h' = (1-z)n + z*mem
  - mask           new_mem = has ? h' : mem
  - community      commT += new_mem_tile.T @ inc_tile  (psum-accumulated)
GRU/time-encoding params are replicated to all cores; the [C,M] community
matmul partials are summed on the host (8 tiny [128,256] tiles).

Engine plan per core: PE runs all matmuls in fp16 (fp32 PSUM accumulate,
~3e-4 overall rel err; set STREAM_DT = f32r for ~1.4e-4 at ~1.3x the time).
ACT does the transcendentals — all Sin ops run in a prologue phase so the
ACT LUT is loaded twice total (trig set, then sigmoid+tanh set) instead of
per-slab. DVE does the psum-coupled GRU gate algebra and the fp16 slab-wide
blend; gate psums are paired two-per-psum-tile so sigmoid/tanh/DVE ops run
at double width. DMA issue is spread over the SP / ACT / Pool sequencers so
no single sequencer serializes the loads. Masked (event-free) nodes are
additionally overwritten on the host with the exact input memory, so fp16
on the blend path never degrades the copy-through rows."""

import numpy as np

from concourse import bacc
import concourse.mybir as mybir
from concourse.tile import TileContext
from concourse.bass_utils import run_bass_kernel_spmd

# Problem shapes (hardcoded per contract).
N, E, C = 100000, 50000, 256
M, D, F, T = 128, 128, 128, 64
NCORES = 8
NPC = N // NCORES          # 12500 nodes per core
SLAB = 1024                # nodes per pipeline slab
P = 128
NTILES = (NPC + P - 1) // P  # 98 node-tiles per core
PAIR = True               # pair sub-tiles in one 2-bank psum tile

f32 = mybir.dt.float32
f32r = mybir.dt.float32r
f16 = mybir.dt.float16
STREAM_DT = f16            # PE-stream dtype: f16 halves DMA bytes (~5e-4 err) vs f32r (~1.5e-4)
NP_STREAM = np.float16 if STREAM_DT == f16 else np.float32
A = mybir.AluOpType
AF = mybir.ActivationFunctionType

_COMPILED = None  # compiled Bacc program cache — build once per process


def _subtiles(w):
    subs = []
    off = 0
    while off < w:
        subs.append((off, min(P, w - off)))
        off += subs[-1][1]
    return subs


def _build_program():
    nc = bacc.Bacc("TRN2", target_bir_lowering=False)

    # Per-core inputs. Feature-major (transposed) copies feed the PE as lhsT;
    # node-major mem/inc feed the blend and the community matmul.
    sT = nc.dram_tensor("sT", [D, NPC], STREAM_DT, kind="ExternalInput")
    dT = nc.dram_tensor("dT", [D, NPC], STREAM_DT, kind="ExternalInput")
    fT = nc.dram_tensor("fT", [F, NPC], STREAM_DT, kind="ExternalInput")
    mT = nc.dram_tensor("mT", [M, NPC], STREAM_DT, kind="ExternalInput")
    yT = nc.dram_tensor("yT", [T + 1, NPC], f32, kind="ExternalInput")  # sin args; row T = pi/2 -> 1.0 bias lane
    mem = nc.dram_tensor("mem", [NPC, M], f16, kind="ExternalInput")
    inc = nc.dram_tensor("inc", [NPC, C], STREAM_DT, kind="ExternalInput")
    hasp = nc.dram_tensor("hasp", [P, NTILES], f32, kind="ExternalInput")
    omhp = nc.dram_tensor("omhp", [P, NTILES], f32, kind="ExternalInput")
    Wa = nc.dram_tensor("Wa", [D, 3 * M], STREAM_DT, kind="ExternalInput")
    Wb = nc.dram_tensor("Wb", [D, 3 * M], STREAM_DT, kind="ExternalInput")
    Wc = nc.dram_tensor("Wc", [F, 3 * M], STREAM_DT, kind="ExternalInput")
    Wt = nc.dram_tensor("Wt", [T + 1, 3 * M], STREAM_DT, kind="ExternalInput")  # row T = b_ih + b_hh
    Whh = nc.dram_tensor("Whh", [M, 3 * M], STREAM_DT, kind="ExternalInput")

    om = nc.dram_tensor("om", [NPC, M], f16, kind="ExternalOutput")
    ocm = nc.dram_tensor("ocm", [M, C], f32, kind="ExternalOutput")

    # Slab schedule: big slabs for DMA efficiency, tapered tail to shrink
    # the end-of-kernel drain chain.
    slabs = [(0, 512), (512, 512)]
    g0 = 1024
    while g0 < NPC:
        rem = NPC - g0
        w = SLAB if rem >= SLAB + 1236 else (512 if rem > 512 else rem)
        slabs.append((g0, w))
        g0 += w

    with TileContext(nc) as tc:
        with tc.tile_pool(name="const", bufs=1) as cpool, \
             tc.tile_pool(name="small", bufs=6) as spool, \
             tc.tile_pool(name="ps", bufs=3 if PAIR else 6, space="PSUM") as pspool, \
             tc.tile_pool(name="pacc", bufs=1, space="PSUM") as paccpool:

            # Persistent tiles
            wa_t = cpool.tile([D, 3 * M], STREAM_DT)
            nc.sync.dma_start(wa_t[:], Wa[:])
            wb_t = cpool.tile([D, 3 * M], STREAM_DT)
            nc.sync.dma_start(wb_t[:], Wb[:])
            wc_t = cpool.tile([F, 3 * M], STREAM_DT)
            nc.sync.dma_start(wc_t[:], Wc[:])
            wt_t = cpool.tile([T + 1, 3 * M], STREAM_DT)
            nc.sync.dma_start(wt_t[:], Wt[:])
            whh_t = cpool.tile([M, 3 * M], STREAM_DT)
            nc.sync.dma_start(whh_t[:], Whh[:])
            has_t = cpool.tile([P, NTILES], f32)
            nc.sync.dma_start(has_t[:], hasp[:])
            omh_t = cpool.tile([P, NTILES], f32)
            nc.sync.dma_start(omh_t[:], omhp[:])

            comm_acc = paccpool.tile([M, C], f32)
            tencr = cpool.tile([T + 1, NPC], STREAM_DT)

            def emit_tenc(chunk):
                g0, w = chunk
                yT_s = ytpool.tile([T + 1, w], f32, tag="yT")
                nc.sync.dma_start(yT_s[:], yT[:, g0:g0 + w])
                nc.scalar.activation(tencr[:, g0:g0 + w], yT_s[:], AF.Sin)

            state = {"tile_idx": 0}

            def emit_slab(chunk):
                g0, w = chunk
                subs = _subtiles(w)
                nsub = len(subs)

                sT_s = pool.tile([D, w], STREAM_DT, tag="sT")
                nc.sync.dma_start(sT_s[:], sT[:, g0:g0 + w])
                dT_s = pool.tile([D, w], STREAM_DT, tag="dT")
                nc.sync.dma_start(dT_s[:], dT[:, g0:g0 + w])
                fT_s = pool.tile([F, w], STREAM_DT, tag="fT")
                nc.sync.dma_start(fT_s[:], fT[:, g0:g0 + w])
                mT_s = pool.tile([M, w], STREAM_DT, tag="mT")
                nc.sync.dma_start(mT_s[:], mT[:, g0:g0 + w])

                mem_s = pool.tile([P, nsub, M], f16, tag="mem")
                inc_s = incpool.tile([P, nsub, C], STREAM_DT, tag="inc")
                if w % P == 0:
                    nc.scalar.dma_start(
                        mem_s[:, 0:w // P, :], mem[g0:g0 + w].rearrange("(s p) f -> p s f", p=P))
                    nc.scalar.dma_start(
                        inc_s[:, 0:w // P, :], inc[g0:g0 + w].rearrange("(s p) f -> p s f", p=P))
                else:
                    for s, (soff, sw) in enumerate(subs):
                        nc.scalar.dma_start(mem_s[:sw, s, :], mem[g0 + soff:g0 + soff + sw])
                        nc.scalar.dma_start(inc_s[:sw, s, :], inc[g0 + soff:g0 + soff + sw])

                n_sl = pool.tile([P, nsub, M], f16, tag="n_sl")
                zp_sl = pool.tile([P, nsub, M], f16, tag="zp_sl")

                # Pair sub-tiles into one 2-bank psum tile so the sigmoid /
                # tanh / DVE gate algebra runs at double width (halves per-op
                # overhead). Ragged tail tiles fall back to single width.
                groups = []
                i = 0
                while i < nsub:
                    if PAIR and i + 1 < nsub and subs[i][1] == P and subs[i + 1][1] == P:
                        groups.append((i, 2))
                        i += 2
                    else:
                        groups.append((i, 1))
                        i += 1

                for (i0, gn) in groups:
                  if gn == 1:
                    s = i0
                    soff, sw = subs[s]
                    gt = state["tile_idx"]
                    pzf = pspool.tile([P, 4 * M], f32, tag="pz")
                    pall = pzf[:, 0:3 * M]
                    phn = pzf[:, 3 * M:4 * M]
                    sl = slice(soff, soff + sw)
                    gsl = slice(g0 + soff, g0 + soff + sw)
                    nc.tensor.matmul(pall[:sw, :], sT_s[:, sl], wa_t[:], start=True, stop=False)
                    nc.tensor.matmul(pall[:sw, :], dT_s[:, sl], wb_t[:], start=False, stop=False)
                    nc.tensor.matmul(pall[:sw, :], fT_s[:, sl], wc_t[:], start=False, stop=False)
                    nc.tensor.matmul(pall[:sw, :], tencr[:, gsl], wt_t[:], start=False, stop=False)
                    nc.tensor.matmul(pall[:sw, 0:2 * M], mT_s[:, sl], whh_t[:, 0:2 * M],
                                     start=False, stop=True)
                    nc.tensor.matmul(phn[:sw, :], mT_s[:, sl], whh_t[:, 2 * M:3 * M],
                                     start=True, stop=True)
                    rz = spool.tile([P, 2 * M], f32, tag="rz")
                    nc.scalar.activation(rz[:sw, :], pall[:sw, 0:2 * M], AF.Sigmoid)
                    tt = spool.tile([P, M], f32, tag="tt")
                    nc.vector.tensor_tensor(tt[:sw, :], rz[:sw, 0:M], phn[:sw, :], A.mult)
                    npre = spool.tile([P, M], f32, tag="npre")
                    nc.vector.tensor_tensor(npre[:sw, :], pall[:sw, 2 * M:3 * M], tt[:sw, :],
                                            A.add)
                    nc.scalar.activation(n_sl[:sw, s, :], npre[:sw, :], AF.Tanh)
                    nc.vector.tensor_scalar(zp_sl[:sw, s, :], rz[:sw, M:2 * M],
                                            has_t[:sw, gt:gt + 1], omh_t[:sw, gt:gt + 1],
                                            A.mult, A.add)
                    state["tile_idx"] += 1
                  else:
                    pz = pspool.tile([P, 2, 4 * M], f32, tag="pz")
                    for j in range(gn):
                        s = i0 + j
                        soff, sw = subs[s]
                        sl = slice(soff, soff + sw)
                        gsl = slice(g0 + soff, g0 + soff + sw)
                        pall = pz[:, j, 0:3 * M]
                        phn = pz[:, j, 3 * M:4 * M]
                        nc.tensor.matmul(pall[:sw, :], sT_s[:, sl], wa_t[:], start=True, stop=False)
                        nc.tensor.matmul(pall[:sw, :], dT_s[:, sl], wb_t[:], start=False, stop=False)
                        nc.tensor.matmul(pall[:sw, :], fT_s[:, sl], wc_t[:], start=False, stop=False)
                        nc.tensor.matmul(pall[:sw, :], tencr[:, gsl], wt_t[:], start=False, stop=False)
                        nc.tensor.matmul(pall[:sw, 0:2 * M], mT_s[:, sl], whh_t[:, 0:2 * M],
                                         start=False, stop=True)
                        nc.tensor.matmul(phn[:sw, :], mT_s[:, sl], whh_t[:, 2 * M:3 * M],
                                         start=True, stop=True)

                    sw0 = subs[i0][1]
                    rz = spool.tile([P, 2, 2 * M], f32, tag="rz")
                    nc.scalar.activation(rz[:sw0, 0:gn, :], pz[:sw0, 0:gn, 0:2 * M], AF.Sigmoid)
                    tt = spool.tile([P, 2, M], f32, tag="tt")
                    nc.vector.tensor_tensor(tt[:sw0, 0:gn, :], rz[:sw0, 0:gn, 0:M],
                                            pz[:sw0, 0:gn, 3 * M:4 * M], A.mult)
                    npre = spool.tile([P, 2, M], f32, tag="npre")
                    nc.vector.tensor_tensor(npre[:sw0, 0:gn, :], pz[:sw0, 0:gn, 2 * M:3 * M],
                                            tt[:sw0, 0:gn, :], A.add)
                    nc.scalar.activation(n_sl[:sw0, i0:i0 + gn, :], npre[:sw0, 0:gn, :], AF.Tanh)
                    for j in range(gn):
                        gt = state["tile_idx"]
                        sw = subs[i0 + j][1]
                        nc.vector.tensor_scalar(zp_sl[:sw, i0 + j, :], rz[:sw, j, M:2 * M],
                                                has_t[:sw, gt:gt + 1], omh_t[:sw, gt:gt + 1],
                                                A.mult, A.add)
                        state["tile_idx"] += 1

                # Slab-level GRU blend (f16, DVE): out = n + z'*(mem - n)
                d_sl = pool.tile([P, nsub, M], f16, tag="d_sl")
                nc.vector.tensor_tensor(d_sl[:], mem_s[:], n_sl[:], A.subtract)
                nc.vector.tensor_tensor(d_sl[:], d_sl[:], zp_sl[:], A.mult)
                out_sl = pool.tile([P, nsub, M], f16, tag="out_sl")
                nc.vector.tensor_tensor(out_sl[:], n_sl[:], d_sl[:], A.add)

                # Community partial: commT[M, C] += new_mem_tile.T @ inc_tile
                base = state["tile_idx"] - nsub
                for s, (soff, sw) in enumerate(subs):
                    nc.tensor.matmul(comm_acc[:], out_sl[:sw, s, :], inc_s[:sw, s, :],
                                     start=(base + s == 0), stop=(base + s == NTILES - 1))

                if w % P == 0:
                    nc.gpsimd.dma_start(
                        om[g0:g0 + w].rearrange("(s p) f -> p s f", p=P), out_sl[:, 0:w // P, :])
                else:
                    for s, (soff, sw) in enumerate(subs):
                        nc.gpsimd.dma_start(om[g0 + soff:g0 + soff + sw], out_sl[:sw, s, :])

            # Sin prologue split: the first two chunks run before slab 0 (so
            # slab 0's sigmoid only waits on two Sins), the rest right after.
            # ACT is in-order, so this gives 4 LUT loads total and an early start.
            with tc.tile_pool(name="ytp", bufs=4) as ytpool, \
                 tc.tile_pool(name="slab", bufs=3) as pool, \
                 tc.tile_pool(name="incp", bufs=2) as incpool:
                ychunks = [(i * 2500, 2500) for i in range(5)]
                for ch in ychunks:
                    emit_tenc(ch)
                for ch in slabs:
                    emit_slab(ch)

                cm = spool.tile([M, C], f32)
                nc.scalar.activation(cm[:], comm_acc[:], AF.Copy)
                nc.sync.dma_start(ocm[:], cm[:])

    nc.compile()
    return nc


def _get_program():
    global _COMPILED
    if _COMPILED is None:
        _COMPILED = _build_program()
    return _COMPILED


def kernel(src, dst, t, last_update, event_feat, src_embeds, dst_embeds,
           nodes_memory, incidence, w_time, b_time, W_ih, W_hh, b_ih, b_hh):
    src = np.asarray(src); dst = np.asarray(dst); t = np.asarray(t)
    last_update = np.asarray(last_update)
    event_feat = np.asarray(event_feat, np.float32)
    src_embeds = np.asarray(src_embeds, np.float32)
    dst_embeds = np.asarray(dst_embeds, np.float32)
    nodes_memory = np.asarray(nodes_memory, np.float32)
    incidence = np.asarray(incidence, np.float32)
    w_time = np.asarray(w_time, np.float32); b_time = np.asarray(b_time, np.float32)
    W_ih = np.asarray(W_ih, np.float32); W_hh = np.asarray(W_hh, np.float32)
    b_ih = np.asarray(b_ih, np.float32); b_hh = np.asarray(b_hh, np.float32)

    # ---- Host routing: 'last' aggregation = stable-sort scatter (index-only) ----
    src_all = np.concatenate([src, dst])
    t_all = np.concatenate([t, t])
    perm = np.argsort(t_all, kind="stable")
    win = np.zeros(N, np.int64)
    win[src_all[perm]] = perm          # ascending rank; last write = newest event
    has = np.bincount(src_all, minlength=N) > 0

    dt_ev = t_all - last_update[src_all]      # int32, per event
    dtw = dt_ev[win].astype(np.float32)       # [N]

    # cos arg, fp32 two-step (matches reference rounding), then f64 range
    # reduction to the Sin-LUT domain: cos(x) = sin(x + pi/2 mod 2pi)
    x = dtw[:, None] * w_time[None, :] + b_time[None, :]
    z = x.astype(np.float64) + (np.pi / 2)
    yp = (z - (2 * np.pi) * np.round(z / (2 * np.pi))).astype(np.float32)  # [N, T]

    # Winner event rows (flipped copies share the original arrays)
    lt = win < E
    ge = ~lt
    w0 = np.where(lt, win, win - E)
    emb_s = np.empty((N, D), np.float32)
    emb_d = np.empty((N, D), np.float32)
    emb_s[lt] = src_embeds[w0[lt]]
    emb_s[ge] = dst_embeds[w0[ge]]
    emb_d[lt] = dst_embeds[w0[lt]]
    emb_d[ge] = src_embeds[w0[ge]]
    feat = event_feat[w0]

    has_f = has.astype(np.float32)

    # Replicated params
    bias_row = (b_ih + b_hh).astype(np.float32)[None, :]
    Wt_ext = np.ascontiguousarray(np.concatenate([W_ih[2 * D + F:], bias_row], axis=0))
    Wa_v = np.ascontiguousarray(W_ih[0:D])
    Wb_v = np.ascontiguousarray(W_ih[D:2 * D])
    Wc_v = np.ascontiguousarray(W_ih[2 * D:2 * D + F])
    Whh_v = np.ascontiguousarray(W_hh)

    pad = NTILES * P - NPC
    in_maps = []
    for c in range(NCORES):
        sl = slice(c * NPC, (c + 1) * NPC)
        yT_c = np.empty((T + 1, NPC), np.float32)
        yT_c[0:T] = yp[sl].T
        yT_c[T] = np.float32(np.pi / 2)      # sin -> 1.0: bias lane
        hp = np.concatenate([has_f[sl], np.zeros(pad, np.float32)])
        hp = np.ascontiguousarray(hp.reshape(NTILES, P).T)
        in_maps.append(dict(
            sT=np.ascontiguousarray(emb_s[sl].T.astype(NP_STREAM)),
            dT=np.ascontiguousarray(emb_d[sl].T.astype(NP_STREAM)),
            fT=np.ascontiguousarray(feat[sl].T.astype(NP_STREAM)),
            mT=np.ascontiguousarray(nodes_memory[sl].T.astype(NP_STREAM)),
            yT=yT_c,
            mem=np.ascontiguousarray(nodes_memory[sl].astype(np.float16)),
            inc=np.ascontiguousarray(incidence[sl].astype(NP_STREAM)),
            hasp=hp,
            omhp=np.ascontiguousarray(1.0 - hp),
            Wa=Wa_v.astype(NP_STREAM), Wb=Wb_v.astype(NP_STREAM), Wc=Wc_v.astype(NP_STREAM), Wt=Wt_ext.astype(NP_STREAM), Whh=Whh_v.astype(NP_STREAM),
        ))

    nc = _get_program()
    res = run_bass_kernel_spmd(nc, in_maps, core_ids=list(range(NCORES)))

    out = np.empty((N + C, M), np.float32)
    comm = np.zeros((M, C), np.float64)
    for c in range(NCORES):
        out[c * NPC:(c + 1) * NPC] = res.results[c]["om"].astype(np.float32)
        comm += res.results[c]["ocm"]
    out[:N][~has] = nodes_memory[~has]   # masked nodes copy memory exactly
    out[N:] = comm.T.astype(np.float32)
    return out



# revision 9
# speedup vs baseline: 1.2850x; 1.2850x over previous
"""Distributed Trainium2 kernel for nn_CONNECT_86964497809993 (TGN-style
GNN message passing: last-event aggregation + GRU memory update + community
incidence matmul), sharded over 8 NeuronCores.

Strategy: event routing ("last message per node") is integer index plumbing
done on the host during input sharding (per the sharding hint); nodes are
then re-partitioned across cores into a has-event set (full GRU pipeline)
and a no-event set (memory passthrough: only the community matmul needs
those rows). Per core:
  - gates   psum = [embs|embd|feat|tenc+mask|mem] @ [W_ih;W_hh]*8 computed
            with fp8e4 DoubleRow matmuls (2 K-tiles per instr, 0.5 cyc/row).
            Each 128-row K-chunk is a DR pair (W8, Wlo) sharing one
            stride-0-broadcast fp8 stream, where W8 = fp8(8W) and
            Wlo = fp8(8W - W8): the fp8 weight-residual kills the
            systematic per-column quantization bias that otherwise
            accumulates over the 100k-node community reduction.
  - mask    has-mask folded into the z-gate as an extra stream row with
            weight 240 (sigma(30) = 1 => passthrough), zero elementwise cost.
  - GRU     ACT: sigmoid/tanh at scale 1/8 (un-scales the 8x fp8 weights);
            DVE: r*hn, xn+, blend adds via scalar_tensor_tensor (4x mode);
            Pool: two blend multiplies (keeps DVE under the slab budget).
  - comm    new_mem tiles PE-transposed (f16, via identity) into PSUM,
            evacuated by DVE tensor_copy, then f16-stationary x fp8-moving
            matmuls accumulate incidence^T partials; no-event nodes
            contribute via fp8 DoubleRow pairs directly from memory tiles.
All streams are feature-major [feat, node] so every DMA moves >=1KB
contiguous runs per partition (full DMA bus rate) and memory is loaded
once. Community partials ([128,256] per core) are summed on the host."""

import numpy as np
import ml_dtypes

from concourse import bacc
import concourse.mybir as mybir
from concourse.tile import TileContext
from concourse.bass_utils import run_bass_kernel_spmd

N, E, C = 100000, 50000, 256
M, D, F, T = 128, 128, 128, 64
NCORES = 8
P = 128

f32 = mybir.dt.float32
f16 = mybir.dt.float16
f8 = mybir.dt.float8e4
A = mybir.AluOpType
AF = mybir.ActivationFunctionType
DR = mybir.MatmulPerfMode.DoubleRow

NP_E4 = ml_dtypes.float8_e4m3
WS = 8.0                  # weight pre-scale (un-done by ACT scale=1/8)
ZBIG = 240.0              # mask weight: sigma(240/8) = 1.0
DMA_SLAB = 2048           # has1 DMA slab (columns)
SUB = 512                 # compute sub-slab (columns)

_COMPILED = {}            # (H1, H0) -> compiled program


def _q8(a):
    return np.asarray(a, np.float32).astype(NP_E4)


def _build_program(H1, H0):
    T1 = H1 // P
    P0 = H0 // 256
    nc = bacc.Bacc("TRN2", target_bir_lowering=False)

    T0 = H0 // P
    X8 = nc.dram_tensor("X8", [P, 5, H1], f8, kind="ExternalInput")
    mT = nc.dram_tensor("mT", [P, H1], f16, kind="ExternalInput")
    inc1 = nc.dram_tensor("inc1", [P, T1, 256], f8, kind="ExternalInput")
    m0 = nc.dram_tensor("m0", [P, T0, 128], f8, kind="ExternalInput")
    inc0 = nc.dram_tensor("inc0", [P, T0, 256], f8, kind="ExternalInput")
    WP = nc.dram_tensor("WP", [P, 15, 2, 128], f8, kind="ExternalInput")
    idt = nc.dram_tensor("idt", [P, P], f16, kind="ExternalInput")
    om = nc.dram_tensor("om", [P, H1], f16, kind="ExternalOutput")
    ocm = nc.dram_tensor("ocm", [P, 256], f32, kind="ExternalOutput")

    # (pair slots, stream planes) per gate segment; pair j multiplies the
    # stride-0-duplicated plane by (W8, Wlo) k-tiles.
    SEGS = [
        (0, [0, 1, 2, 3, 4], [0, 1, 2, 3, 4]),   # r   <- psum rz[:,0,:]
        (1, [5, 6, 7, 8, 9], [0, 1, 2, 3, 4]),   # z   <- psum rz[:,1,:]
        (2, [10, 11, 12, 13], [0, 1, 2, 3]),     # xn  <- psum xn[:,0,:]
        (3, [14], [4]),                          # hn  <- psum xn[:,1,:]
    ]

    with TileContext(nc) as tc:
        with tc.tile_pool(name="const", bufs=1) as cpool, \
             tc.tile_pool(name="h0p", bufs=2) as h0pool, \
             tc.tile_pool(name="xp", bufs=2) as xpool, \
             tc.tile_pool(name="gp", bufs=3) as gpool, \
             tc.tile_pool(name="op", bufs=2) as opool, \
             tc.tile_pool(name="rzp", bufs=2, space="PSUM") as rzpool, \
             tc.tile_pool(name="xnp", bufs=1, space="PSUM") as xnpool, \
             tc.tile_pool(name="trp", bufs=1, space="PSUM") as trpool, \
             tc.tile_pool(name="cap", bufs=1, space="PSUM") as capool:

            wp_t = cpool.tile([P, 15, 2, 128], f8)
            nc.scalar.dma_start(wp_t[:], WP[:])
            id_t = cpool.tile([P, P], f16)
            nc.scalar.dma_start(id_t[:], idt[:])
            comm = capool.tile([P, 256], f32)

            # ---- Phase 0: no-event nodes -> comm += mem_tile^T @ inc ----
            m0_t = cpool.tile([P, T0, 128], f8)
            nc.scalar.dma_start(m0_t[:], m0[:])
            inc0_t = cpool.tile([P, T0, 256], f8)
            nc.scalar.dma_start(inc0_t[:], inc0[:])
            for p in range(T0):
                nc.tensor.matmul(comm[:], m0_t[:, p, :], inc0_t[:, p, :],
                                 start=(p == 0), stop=False)

            # ---- Phase 1: has-event nodes, slab pipeline ----
            for g0 in range(0, H1, DMA_SLAB):
                W = min(DMA_SLAB, H1 - g0)
                x_s = xpool.tile([P, 5, W], f8, tag="x")
                nc.sync.dma_start(x_s[:], X8[:, :, g0:g0 + W])
                mT_s = xpool.tile([P, W], f16, tag="mT")
                nc.sync.dma_start(mT_s[:], mT[:, g0:g0 + W])
                inc_s = xpool.tile([P, W // P, 256], f8, tag="inc")
                nc.gpsimd.dma_start(inc_s[:], inc1[:, g0 // P:(g0 + W) // P, :])
                o_s = opool.tile([P, W], f16, tag="o")

                for s0 in range(0, W, SUB):
                    rz_ps = rzpool.tile([P, 2, SUB], f32, tag="rz")
                    xn_ps = xnpool.tile([P, 2, SUB], f32, tag="xn")
                    for h0c in range(0, SUB, 256):
                        cs = slice(s0 + h0c, s0 + h0c + 256)
                        rhs = [x_s[:, pl, cs].unsqueeze(1).broadcast_to([P, 2, 256])
                               for pl in range(5)]
                        for d_idx, pairs, planes in SEGS:
                            dest = (rz_ps if d_idx < 2 else xn_ps)[
                                :, d_idx % 2, h0c:h0c + 256]
                            npair = len(pairs)
                            for i, (j, pl) in enumerate(zip(pairs, planes)):
                                nc.tensor.matmul(dest, wp_t[:, j, :, :], rhs[pl],
                                                 start=(i == 0),
                                                 stop=(i == npair - 1),
                                                 perf_mode=DR)
                    rz = gpool.tile([P, 2, SUB], f16, tag="rzs")
                    nc.scalar.activation(rz[:], rz_ps[:], AF.Sigmoid, scale=1.0 / WS)
                    tt = gpool.tile([P, SUB], f16, tag="tt")
                    nc.vector.scalar_tensor_tensor(tt[:], rz[:, 0, :], 1.0,
                                                   xn_ps[:, 1, :], A.mult, A.mult)
                    npre = gpool.tile([P, SUB], f16, tag="npre")
                    nc.vector.tensor_tensor(npre[:], xn_ps[:, 0, :], tt[:], A.add)
                    n_t = gpool.tile([P, SUB], f16, tag="n")
                    nc.scalar.activation(n_t[:], npre[:], AF.Tanh, scale=1.0 / WS)
                    d_t = gpool.tile([P, SUB], f16, tag="d")
                    nc.gpsimd.tensor_tensor(d_t[:], mT_s[:, s0:s0 + SUB],
                                            n_t[:], A.subtract)
                    e_t = gpool.tile([P, SUB], f16, tag="e")
                    nc.gpsimd.tensor_tensor(e_t[:], rz[:, 1, :], d_t[:], A.mult)
                    o_sub = o_s[:, s0:s0 + SUB]
                    nc.vector.scalar_tensor_tensor(o_sub, n_t[:], 1.0,
                                                   e_t[:], A.mult, A.add)

                    tr_ps = trpool.tile([P, 4, 128], f16, tag="tr")
                    for k in range(SUB // P):
                        nc.tensor.transpose(tr_ps[:, k, :],
                                            o_s[:, s0 + P * k:s0 + P * (k + 1)], id_t[:])
                    nmT = gpool.tile([P, 4, 128], f16, tag="nmT")
                    nc.vector.tensor_copy(nmT[:], tr_ps[:])
                    for k in range(SUB // P):
                        t_idx = (g0 + s0) // P + k
                        nc.tensor.matmul(comm[:], nmT[:, k, :], inc_s[:, s0 // P + k, :],
                                         start=False, stop=(t_idx == T1 - 1))
                nc.sync.dma_start(om[:, g0:g0 + W], o_s[:])

            cm = gpool.tile([P, 256], f32, tag="cm")
            nc.scalar.activation(cm[:], comm[:], AF.Copy)
            nc.sync.dma_start(ocm[:], cm[:])

    nc.compile()
    return nc


def _get_program(H1, H0):
    key = (H1, H0)
    if key not in _COMPILED:
        _COMPILED[key] = _build_program(H1, H0)
    return _COMPILED[key]


def _ceil_to(x, q):
    return (x + q - 1) // q * q


def kernel(src, dst, t, last_update, event_feat, src_embeds, dst_embeds,
           nodes_memory, incidence, w_time, b_time, W_ih, W_hh, b_ih, b_hh):
    src = np.asarray(src); dst = np.asarray(dst); t = np.asarray(t)
    last_update = np.asarray(last_update)
    event_feat = np.asarray(event_feat, np.float32)
    src_embeds = np.asarray(src_embeds, np.float32)
    dst_embeds = np.asarray(dst_embeds, np.float32)
    nodes_memory = np.asarray(nodes_memory, np.float32)
    incidence = np.asarray(incidence, np.float32)
    w_time = np.asarray(w_time, np.float32); b_time = np.asarray(b_time, np.float32)
    W_ih = np.asarray(W_ih, np.float32); W_hh = np.asarray(W_hh, np.float32)
    b_ih = np.asarray(b_ih, np.float32); b_hh = np.asarray(b_hh, np.float32)

    # ---- Host routing: 'last' aggregation = stable-sort scatter (index-only) ----
    src_all = np.concatenate([src, dst])
    t_all = np.concatenate([t, t])
    perm = np.argsort(t_all, kind="stable")
    win = np.zeros(N, np.int64)
    win[src_all[perm]] = perm          # ascending rank; last write = newest event
    has = np.bincount(src_all, minlength=N) > 0

    nodes1 = np.where(has)[0]
    nodes0 = np.where(~has)[0]
    n1, n0 = len(nodes1), len(nodes0)
    h1c = (n1 + NCORES - 1) // NCORES
    h0c = (n0 + NCORES - 1) // NCORES
    H1 = _ceil_to(max(h1c, 1), SUB)
    H0 = _ceil_to(max(h0c, 1), 256)
    T0 = H0 // P

    # Winner-event data for has-event nodes
    w1 = win[nodes1]
    dtw = (t_all[w1] - last_update[nodes1]).astype(np.float32)
    tenc = np.cos(dtw[:, None] * w_time[None, :] + b_time[None, :])  # [n1, T]
    lt = w1 < E
    w0 = np.where(lt, w1, w1 - E)
    emb_s = np.where(lt[:, None], src_embeds[w0], dst_embeds[w0])
    emb_d = np.where(lt[:, None], dst_embeds[w0], src_embeds[w0])
    feat = event_feat[w0]

    # ---- Replicated weights: fp8 DoubleRow pairs (W8, Wlo), pre-scaled by 8 ----
    bias = (b_ih + b_hh).astype(np.float32)
    chunks = {}  # name -> [128, 384] f32 weight rows
    chunks['A'] = W_ih[0:128]
    chunks['B'] = W_ih[128:256]
    chunks['C'] = W_ih[256:384]
    Dr = np.zeros((128, 384), np.float32)
    Dr[0:T] = W_ih[384:448]
    Dr[T + 1] = bias                      # bias lane (stream row = 1.0)
    chunks['D'] = Dr
    chunks['E'] = W_hh
    # slot layout: r(seg0): A..E, z(seg1): A..E, xn(seg2): A..D, hn(seg2): E
    WPa = np.zeros((P, 15, 2, 128), np.float32)
    layout = ([(0, c) for c in "ABCDE"] + [(1, c) for c in "ABCDE"] +
              [(2, c) for c in "ABCD"] + [(2, 'E')])
    for j, (seg, cname) in enumerate(layout):
        wseg = chunks[cname][:, 128 * seg:128 * (seg + 1)] * WS
        if seg == 1 and cname == 'D':   # z-seg D chunk carries the has-mask row
            wseg = wseg.copy()
            wseg[T] = ZBIG
        hi = _q8(wseg).astype(np.float32)
        lo = _q8(wseg - hi).astype(np.float32)
        WPa[:, j, 0, :] = hi
        WPa[:, j, 1, :] = lo
    WP_v = WPa.astype(NP_E4)

    ident = np.eye(P, dtype=np.float16)

    nc = _get_program(H1, H0)

    in_maps = []
    core_n1 = []
    for c in range(NCORES):
        i0, i1 = c * h1c, min((c + 1) * h1c, n1)
        cn1 = max(i1 - i0, 0)
        core_n1.append((i0, i1))
        sl = slice(i0, i1)
        X8c = np.zeros((P, 5, H1), NP_E4)
        X8c[:, 0, :cn1] = _q8(emb_s[sl].T)
        X8c[:, 1, :cn1] = _q8(emb_d[sl].T)
        X8c[:, 2, :cn1] = _q8(feat[sl].T)
        tpl = np.zeros((P, H1), np.float32)
        tpl[0:T, :cn1] = tenc[sl].T
        tpl[T, cn1:] = 1.0                # mask row: 1 on padding columns
        tpl[T + 1, :] = 1.0               # bias lane
        X8c[:, 3, :] = _q8(tpl)
        X8c[:, 4, :cn1] = _q8(nodes_memory[nodes1[sl]].T)
        mTc = np.zeros((P, H1), np.float16)
        mTc[:, :cn1] = nodes_memory[nodes1[sl]].T.astype(np.float16)
        inc1c = np.zeros((H1 // P, P, 256), NP_E4)
        inc1c.reshape(H1, 256)[:cn1] = _q8(incidence[nodes1[sl]])
        inc1c = np.ascontiguousarray(inc1c.transpose(1, 0, 2))

        j0, j1 = c * h0c, min((c + 1) * h0c, n0)
        cn0 = max(j1 - j0, 0)
        sl0 = nodes0[j0:j1]
        m0c = np.zeros((H0, 128), NP_E4)
        m0c[:cn0] = _q8(nodes_memory[sl0])
        m0c = np.ascontiguousarray(m0c.reshape(T0, P, 128).transpose(1, 0, 2))
        inc0c = np.zeros((H0, 256), NP_E4)
        inc0c[:cn0] = _q8(incidence[sl0])
        inc0c = np.ascontiguousarray(inc0c.reshape(T0, P, 256).transpose(1, 0, 2))

        in_maps.append(dict(X8=X8c, mT=mTc, inc1=inc1c, m0=m0c, inc0=inc0c,
                            WP=WP_v, idt=ident))

    res = run_bass_kernel_spmd(nc, in_maps, core_ids=list(range(NCORES)))

    out = np.empty((N + C, M), np.float32)
    out[:N] = nodes_memory
    comm = np.zeros((M, C), np.float64)
    for c in range(NCORES):
        i0, i1 = core_n1[c]
        if i1 > i0:
            out[nodes1[i0:i1]] = res.results[c]["om"][:, :i1 - i0].T.astype(np.float32)
        comm += res.results[c]["ocm"]
    out[N:] = comm.T.astype(np.float32)
    return out


# revision 49
# speedup vs baseline: 2.0400x; 1.5876x over previous
"""Distributed Trainium2 kernel for nn_CONNECT_86964497809993 (TGN-style
GNN message passing: last-event aggregation + GRU memory update + community
incidence matmul), sharded over 8 NeuronCores.

Strategy: event routing ("last message per node") is integer index plumbing
done on the host during input sharding (per the sharding hint); nodes are
then re-partitioned across cores into a has-event set (full GRU pipeline)
and a no-event set (memory passthrough: only the community matmul needs
those rows). Per core:
  - gates   psum = [embs|embd|feat|tenc+mask|mem] @ [W_ih;W_hh]*8 computed
            with fp8e4 DoubleRow matmuls (2 K-tiles per instr, 0.5 cyc/row).
            Each 128-row K-chunk is a DR pair (W8, Wlo) sharing one
            stride-0-broadcast fp8 stream, where W8 = fp8(8W) and
            Wlo = fp8(8W - W8): the fp8 weight-residual kills the
            systematic per-column quantization bias that otherwise
            accumulates over the 100k-node community reduction.
  - mask    has-mask folded into the z-gate as an extra stream row with
            weight 240 (sigma(30) = 1 => passthrough), zero elementwise cost.
  - GRU     ACT: sigmoid/tanh at scale 1/8 (un-scales the 8x fp8 weights);
            DVE: r*hn, xn+, blend adds via scalar_tensor_tensor (4x mode);
            Pool: two blend multiplies (keeps DVE under the slab budget).
  - comm    new_mem tiles PE-transposed (f16, via identity) into PSUM,
            evacuated by DVE tensor_copy, then f16-stationary x fp8-moving
            matmuls accumulate incidence^T partials; no-event nodes
            contribute via fp8 DoubleRow pairs directly from memory tiles.
All streams are feature-major [feat, node] so every DMA moves >=1KB
contiguous runs per partition (full DMA bus rate) and memory is loaded
once. Community partials ([128,256] per core) are summed on the host."""

import numpy as np
import ml_dtypes

from concourse import bacc
import concourse.mybir as mybir
from concourse.tile import TileContext
from concourse.bass_utils import run_bass_kernel_spmd

N, E, C = 100000, 50000, 256
M, D, F, T = 128, 128, 128, 64
NCORES = 8
P = 128

f32 = mybir.dt.float32
f16 = mybir.dt.float16
f8 = mybir.dt.float8e4
f8e3 = mybir.dt.float8e3
A = mybir.AluOpType
AF = mybir.ActivationFunctionType
DR = mybir.MatmulPerfMode.DoubleRow

NP_E4 = ml_dtypes.float8_e4m3
NP_E3 = ml_dtypes.float8_e3m4
WS = 8.0                  # weight pre-scale (un-done by ACT scale=1/8)
ZBIG = 240.0              # mask weight: sigma(240/8) = 1.0
DMA_SLAB = 1024           # has1 DMA slab (columns)
SUB = 512                 # compute sub-slab (columns)

_COMPILED = {}            # (H1, H0) -> compiled program


def _q8(a):
    return np.asarray(a, np.float32).astype(NP_E4)


def _q8e3(a):
    return np.asarray(a, np.float32).astype(NP_E3)


def _build_program(H1, H0):
    T1 = H1 // P
    T0 = H0 // P
    KD = T + 2  # D-chunk contraction depth: 64 tenc + mask + bias
    nc = bacc.Bacc("TRN2", target_bir_lowering=False)

    X8 = nc.dram_tensor("X8", [P, 4, H1], f8, kind="ExternalInput")
    t8 = nc.dram_tensor("t8", [KD, H1], f8, kind="ExternalInput")
    mT = nc.dram_tensor("mT", [P, H1], f16, kind="ExternalInput")
    inc1 = nc.dram_tensor("inc1", [P, T1, 256], f8e3, kind="ExternalInput")
    m0 = nc.dram_tensor("m0", [P, T0, 128], f8e3, kind="ExternalInput")
    inc0 = nc.dram_tensor("inc0", [P, T0, 256], f8e3, kind="ExternalInput")
    WP = nc.dram_tensor("WP", [P, 13, 2, 128], f8, kind="ExternalInput")
    idt = nc.dram_tensor("idt", [P, P], f16, kind="ExternalInput")
    om = nc.dram_tensor("om", [P, H1], f16, kind="ExternalOutput")
    ocm = nc.dram_tensor("ocm", [P, 256], f32, kind="ExternalOutput")

    # X8 planes: 0=emb_s 1=emb_d 2=mem 3=feat; plane 4 = t8 (66-deep chunk).
    # r-gate runs without the weight-residual (numerically validated): its
    # k-tile pairs are adjacent X8 plane pairs (s,d) and (m,f) plus t8, so it
    # needs only 3 DoubleRow instructions. z/xn/hn keep (W8,Wlo) pairs on a
    # stride-0-duplicated stream.
    SEGS = [
        (1, [3, 4, 5, 6, 7], [0, 1, 3, 4, 2]),   # z   <- psum rz[:,1,:]
        (2, [8, 9, 10, 11], [0, 1, 3, 4]),       # xn  <- psum xn[:,0,:]
        (3, [12], [2]),                          # hn  <- psum xn[:,1,:]
    ]

    with TileContext(nc) as tc:
        with tc.tile_pool(name="const", bufs=1) as cpool, \
             tc.tile_pool(name="xp", bufs=3) as xpool, \
             tc.tile_pool(name="gp", bufs=3) as gpool, \
             tc.tile_pool(name="op", bufs=3) as opool, \
             tc.tile_pool(name="rzp", bufs=1, space="PSUM") as rzpool, \
             tc.tile_pool(name="xnp", bufs=2, space="PSUM") as xnpool, \
             tc.tile_pool(name="trp", bufs=1, space="PSUM") as trpool, \
             tc.tile_pool(name="cap", bufs=1, space="PSUM") as capool:

            wp_t = cpool.tile([P, 13, 2, 128], f8)
            nc.scalar.dma_start(wp_t[:, 0:3], WP[:, 0:3])   # r-gate slots first
            nc.scalar.dma_start(wp_t[:, 3:13], WP[:, 3:13])
            id_t = cpool.tile([P, P], f16)
            nc.scalar.dma_start(id_t[:], idt[:])
            comm = capool.tile([P, 256], f32)
            m0_t = cpool.tile([P, T0, 128], f8e3)
            inc0_t = cpool.tile([P, T0, 256], f8e3)

            # PE p-state warm-up: dependency-free matmuls on a zeroed scratch
            # tile keep the tensor engine continuously busy through the first
            # input DMAs so real gate matmuls start at full clock. Results land
            # in the comm psum bank, which the first real community matmul
            # resets via its start flag.
            warm = gpool.tile([P, 512], f16, tag="warm")
            nc.vector.memset(warm[:], 0.0)
            for _ in range(20):
                nc.tensor.matmul(comm[:], warm[:, 0:128], warm[:, 256:512],
                                 start=True, stop=True, skip_group_check=True)

            # ---- Phase 1: has-event nodes ----
            # Fully software-pipelined across in-order engine queues:
            #   cycle k emits: gates(k) [PE], sigma(k) [ACT], tt/npre(k) [DVE],
            #   then the LAGGED stages: tanh(k-1) [ACT] (so sigma(k) is never
            #   queued behind a tanh that waits on DVE), blend(k-1) [Pool+DVE],
            #   transposes(k-3) [PE], evac(k-3) [DVE], comm(k-4) [PE], and a
            #   few phase-0 matmuls to fill PE gaps.
            h0_state = {"next": 0, "armed": False, "dma": False}

            def emit_h0(count):
                # The final phase-0 matmul (stop flag) is reserved for the drain.
                p = h0_state["next"]
                while count > 0 and p < T0 - 1:
                    nc.tensor.matmul(comm[:], m0_t[:, p, :], inc0_t[:, p, :],
                                     start=False, stop=False)
                    p += 1; count -= 1
                h0_state["next"] = p

            # Progressive slab sizes: small first slabs start PE early; later,
            # bigger transfers hide behind compute.
            slab_edges = [0]
            for w in (512, 512, 1024, 1024):
                if slab_edges[-1] + w < H1:
                    slab_edges.append(slab_edges[-1] + w)
            while slab_edges[-1] < H1:
                slab_edges.append(min(slab_edges[-1] + DMA_SLAB, H1))

            # Stage A: gate matmuls + sigma + tt/npre for sub-slab ss.
            def stage_a(ss):
                x_s, t8_s, mT_s, inc_s, o_s, g0, s0 = (
                    ss["x"], ss["t8"], ss["mT"], ss["inc"], ss["o"],
                    ss["g0"], ss["s0"])
                rz_ps = rzpool.tile([P, 2, SUB], f32, tag="rz")
                xn_ps = xnpool.tile([P, 2, SUB], f32, tag="xn")
                for h0c in range(0, SUB, 256):
                    cs = slice(s0 + h0c, s0 + h0c + 256)
                    rhs = [x_s[:, pl, cs].unsqueeze(1).broadcast_to([P, 2, 256])
                           for pl in range(4)]
                    rhs.append(t8_s[:, cs].unsqueeze(1).broadcast_to([KD, 2, 256]))
                    # r-gate: (s,d) and (m,f) adjacent-plane pairs + t8
                    dest = rz_ps[:, 0, h0c:h0c + 256]
                    nc.tensor.matmul(dest, wp_t[:, 0, :, :], x_s[:, 0:2, cs],
                                     start=True, stop=False, perf_mode=DR)
                    nc.tensor.matmul(dest, wp_t[:, 1, :, :], x_s[:, 2:4, cs],
                                     start=False, stop=False, perf_mode=DR)
                    nc.tensor.matmul(dest, wp_t[0:KD, 2, :, :], rhs[4],
                                     start=False, stop=True, perf_mode=DR)
                    for d_idx, pairs, planes in SEGS:
                        dest = (rz_ps if d_idx < 2 else xn_ps)[
                            :, d_idx % 2, h0c:h0c + 256]
                        npair = len(pairs)
                        for i, (j, pl) in enumerate(zip(pairs, planes)):
                            lhs = wp_t[0:KD, j, :, :] if pl == 4 else wp_t[:, j, :, :]
                            nc.tensor.matmul(dest, lhs, rhs[pl],
                                             start=(i == 0), stop=(i == npair - 1),
                                             perf_mode=DR)
                rz = gpool.tile([P, 2, SUB], f16, tag="rzs")
                nc.scalar.activation(rz[:], rz_ps[:], AF.Sigmoid, scale=1.0 / WS)
                tt = gpool.tile([P, SUB], f16, tag="tt")
                nc.vector.tensor_tensor(tt[:], rz[:, 0, :], xn_ps[:, 1, :], A.mult)
                npre = gpool.tile([P, SUB], f16, tag="npre")
                nc.vector.tensor_tensor(npre[:], xn_ps[:, 0, :], tt[:], A.add)
                ss["rz"], ss["npre"] = rz, npre

            # Stage B: tanh + blend for sub-slab ss (one cycle after stage A).
            def stage_b(ss):
                n_t = gpool.tile([P, SUB], f16, tag="n")
                nc.scalar.activation(n_t[:], ss["npre"][:], AF.Tanh, scale=1.0 / WS)
                d_t = gpool.tile([P, SUB], f16, tag="d")
                nc.vector.tensor_tensor(d_t[:], ss["mT"][:, ss["s0"]:ss["s0"] + SUB],
                                        n_t[:], A.subtract)
                e_t = gpool.tile([P, SUB], f16, tag="e")
                nc.vector.tensor_tensor(e_t[:], ss["rz"][:, 1, :], d_t[:], A.mult)
                nc.gpsimd.tensor_tensor(ss["o"][:, ss["s0"]:ss["s0"] + SUB],
                                        n_t[:], e_t[:], A.add)
                if ss["last_in_slab"]:
                    nc.sync.dma_start(om[:, ss["g0"]:ss["g0"] + ss["W"]], ss["o"][:])

            # Stage C: transposes + psum evacuation (lag 3).
            def stage_c(ss):
                tr_ps = trpool.tile([P, 4, 128], f16, tag="tr")
                nmT = gpool.tile([P, 4, 128], f16, tag="nmT")
                for k in range(SUB // P):
                    nc.tensor.transpose(tr_ps[:, k, :],
                                        ss["o"][:, ss["s0"] + P * k:ss["s0"] + P * (k + 1)],
                                        id_t[:])
                nc.vector.tensor_copy(nmT[:], tr_ps[:])
                ss["nmT"] = nmT

            # Stage D: community matmuls (lag 4).
            def stage_d(ss):
                for k in range(SUB // P):
                    t_idx = (ss["g0"] + ss["s0"]) // P + k
                    nc.tensor.matmul(comm[:], ss["nmT"][:, k, :],
                                     ss["inc"][:, ss["s0"] // P + k, :],
                                     start=(t_idx == 0), stop=False)

            subs = []
            emitted = {"b": 0, "c": 0, "d": 0}

            def pump(k):
                # Run lagged stages for cycle k of the pipeline.
                if k - 1 >= 0 and k - 1 < len(subs):
                    stage_b(subs[k - 1]); emitted["b"] = k
                if k - 3 >= 0 and k - 3 < len(subs):
                    stage_c(subs[k - 3]); emitted["c"] = k - 3
                if k - 4 >= 0 and k - 4 < len(subs):
                    stage_d(subs[k - 4]); emitted["d"] = k - 4
                    if h0_state["armed"]:
                        emit_h0(4)

            kk = 0
            for g0, g1 in zip(slab_edges[:-1], slab_edges[1:]):
                W = g1 - g0
                x_s = xpool.tile([P, 4, W], f8, tag="x")
                nc.sync.dma_start(x_s[:], X8[:, :, g0:g0 + W])
                t8_s = xpool.tile([KD, W], f8, tag="t8")
                nc.gpsimd.dma_start(t8_s[:], t8[:, g0:g0 + W])
                mT_s = xpool.tile([P, W], f16, tag="mT")
                nc.scalar.dma_start(mT_s[:], mT[:, g0:g0 + W])
                inc_s = xpool.tile([P, W // P, 256], f8e3, tag="inc")
                nc.gpsimd.dma_start(inc_s[:], inc1[:, g0 // P:(g0 + W) // P, :])
                o_s = opool.tile([P, W], f16, tag="o")
                if g0 == 3072:
                    nc.scalar.dma_start(m0_t[:], m0[:])
                    nc.scalar.dma_start(inc0_t[:], inc0[:])
                    h0_state["dma"] = True
                elif g0 >= 5120:
                    h0_state["armed"] = h0_state["dma"]

                for s0 in range(0, W, SUB):
                    subs.append(dict(x=x_s, t8=t8_s, mT=mT_s, inc=inc_s, o=o_s,
                                     g0=g0, s0=s0, W=W,
                                     last_in_slab=(s0 + SUB >= W)))
                    stage_a(subs[kk])
                    pump(kk)
                    kk += 1

            if not h0_state["dma"]:  # few-slab edge case: load phase-0 now
                nc.scalar.dma_start(m0_t[:], m0[:])
                nc.scalar.dma_start(inc0_t[:], inc0[:])

            # Drain the pipeline.
            for k in range(kk, kk + 5):
                pump(k)

            # ---- Phase 0 remainder (drain) ----
            for p in range(h0_state["next"], T0):
                nc.tensor.matmul(comm[:], m0_t[:, p, :], inc0_t[:, p, :],
                                 start=False, stop=(p == T0 - 1))

            cm = gpool.tile([P, 256], f32, tag="cm")
            nc.scalar.activation(cm[:], comm[:], AF.Copy)
            nc.sync.dma_start(ocm[:], cm[:])

    nc.compile()
    return nc


def _get_program(H1, H0):
    key = (H1, H0)
    if key not in _COMPILED:
        _COMPILED[key] = _build_program(H1, H0)
    return _COMPILED[key]


def _ceil_to(x, q):
    return (x + q - 1) // q * q


def kernel(src, dst, t, last_update, event_feat, src_embeds, dst_embeds,
           nodes_memory, incidence, w_time, b_time, W_ih, W_hh, b_ih, b_hh):
    src = np.asarray(src); dst = np.asarray(dst); t = np.asarray(t)
    last_update = np.asarray(last_update)
    event_feat = np.asarray(event_feat, np.float32)
    src_embeds = np.asarray(src_embeds, np.float32)
    dst_embeds = np.asarray(dst_embeds, np.float32)
    nodes_memory = np.asarray(nodes_memory, np.float32)
    incidence = np.asarray(incidence, np.float32)
    w_time = np.asarray(w_time, np.float32); b_time = np.asarray(b_time, np.float32)
    W_ih = np.asarray(W_ih, np.float32); W_hh = np.asarray(W_hh, np.float32)
    b_ih = np.asarray(b_ih, np.float32); b_hh = np.asarray(b_hh, np.float32)

    # ---- Host routing: 'last' aggregation = stable-sort scatter (index-only) ----
    src_all = np.concatenate([src, dst])
    t_all = np.concatenate([t, t])
    perm = np.argsort(t_all, kind="stable")
    win = np.zeros(N, np.int64)
    win[src_all[perm]] = perm          # ascending rank; last write = newest event
    has = np.bincount(src_all, minlength=N) > 0

    nodes1 = np.where(has)[0]
    nodes0 = np.where(~has)[0]
    n1, n0 = len(nodes1), len(nodes0)
    h1c = (n1 + NCORES - 1) // NCORES
    h0c = (n0 + NCORES - 1) // NCORES
    H1 = _ceil_to(max(h1c, 1), SUB)
    H0 = _ceil_to(max(h0c, 1), 256)
    T0 = H0 // P

    # Winner-event data for has-event nodes
    w1 = win[nodes1]
    dtw = (t_all[w1] - last_update[nodes1]).astype(np.float32)
    tenc = np.cos(dtw[:, None] * w_time[None, :] + b_time[None, :])  # [n1, T]
    lt = w1 < E
    w0 = np.where(lt, w1, w1 - E)
    emb_s = np.where(lt[:, None], src_embeds[w0], dst_embeds[w0])
    emb_d = np.where(lt[:, None], dst_embeds[w0], src_embeds[w0])
    feat = event_feat[w0]

    # ---- Replicated weights: fp8 DoubleRow pairs (W8, Wlo), pre-scaled by 8 ----
    bias = (b_ih + b_hh).astype(np.float32)
    chunks = {}  # name -> [128, 384] f32 weight rows
    chunks['A'] = W_ih[0:128]
    chunks['B'] = W_ih[128:256]
    chunks['C'] = W_ih[256:384]
    Dr = np.zeros((128, 384), np.float32)
    Dr[0:T] = W_ih[384:448]
    Dr[T + 1] = bias                      # bias lane (stream row = 1.0)
    chunks['D'] = Dr
    chunks['E'] = W_hh
    # Slot layout: r (slots 0-2, no residual): [A8|B8], [E8|C8], [D8|0];
    # z (3-7), xn (8-11), hn (12): (W8, Wlo) residual pairs per chunk.
    WPa = np.zeros((P, 13, 2, 128), np.float32)

    def _hi(cname, seg):
        wseg = chunks[cname][:, 128 * seg:128 * (seg + 1)] * WS
        if seg == 1 and cname == 'D':   # z-seg D chunk carries the has-mask row
            wseg = wseg.copy()
            wseg[T] = ZBIG
        return wseg, _q8(wseg).astype(np.float32)

    WPa[:, 0, 0, :] = _hi('A', 0)[1]
    WPa[:, 0, 1, :] = _hi('B', 0)[1]
    WPa[:, 1, 0, :] = _hi('E', 0)[1]
    WPa[:, 1, 1, :] = _hi('C', 0)[1]
    WPa[:, 2, 0, :] = _hi('D', 0)[1]
    for j, (seg, cname) in enumerate([(1, c) for c in "ABCDE"] +
                                     [(2, c) for c in "ABCD"] + [(2, 'E')], start=3):
        wseg, hi = _hi(cname, seg)
        WPa[:, j, 0, :] = hi
        WPa[:, j, 1, :] = _q8(wseg - hi).astype(np.float32)
    WP_v = WPa.astype(NP_E4)

    ident = np.eye(P, dtype=np.float16)

    nc = _get_program(H1, H0)

    in_maps = []
    core_n1 = []
    for c in range(NCORES):
        i0, i1 = c * h1c, min((c + 1) * h1c, n1)
        cn1 = max(i1 - i0, 0)
        core_n1.append((i0, i1))
        sl = slice(i0, i1)
        X8c = np.zeros((P, 4, H1), NP_E4)
        X8c[:, 0, :cn1] = _q8(emb_s[sl].T)
        X8c[:, 1, :cn1] = _q8(emb_d[sl].T)
        X8c[:, 2, :cn1] = _q8(nodes_memory[nodes1[sl]].T)
        X8c[:, 3, :cn1] = _q8(feat[sl].T)
        tpl = np.zeros((T + 2, H1), np.float32)
        tpl[0:T, :cn1] = tenc[sl].T
        tpl[T, cn1:] = 1.0                # mask row: 1 on padding columns
        tpl[T + 1, :] = 1.0               # bias lane
        t8c = _q8(tpl)
        mTc = np.zeros((P, H1), np.float16)
        mTc[:, :cn1] = nodes_memory[nodes1[sl]].T.astype(np.float16)
        inc1c = np.zeros((H1 // P, P, 256), NP_E3)
        inc1c.reshape(H1, 256)[:cn1] = _q8e3(incidence[nodes1[sl]])
        inc1c = np.ascontiguousarray(inc1c.transpose(1, 0, 2))

        j0, j1 = c * h0c, min((c + 1) * h0c, n0)
        cn0 = max(j1 - j0, 0)
        sl0 = nodes0[j0:j1]
        m0c = np.zeros((H0, 128), NP_E3)
        m0c[:cn0] = _q8e3(nodes_memory[sl0])
        m0c = np.ascontiguousarray(m0c.reshape(T0, P, 128).transpose(1, 0, 2))
        inc0c = np.zeros((H0, 256), NP_E3)
        inc0c[:cn0] = _q8e3(incidence[sl0])
        inc0c = np.ascontiguousarray(inc0c.reshape(T0, P, 256).transpose(1, 0, 2))

        in_maps.append(dict(X8=X8c, t8=t8c, mT=mTc, inc1=inc1c, m0=m0c, inc0=inc0c,
                            WP=WP_v, idt=ident))

    res = run_bass_kernel_spmd(nc, in_maps, core_ids=list(range(NCORES)))

    out = np.empty((N + C, M), np.float32)
    out[:N] = nodes_memory
    comm = np.zeros((M, C), np.float64)
    for c in range(NCORES):
        i0, i1 = core_n1[c]
        if i1 > i0:
            out[nodes1[i0:i1]] = res.results[c]["om"][:, :i1 - i0].T.astype(np.float32)
        comm += res.results[c]["ocm"]
    out[N:] = comm.T.astype(np.float32)
    return out
